# revision 17
# baseline (speedup 1.0000x reference)
"""ErrorAwareEdgeLoss Trainium2 kernel.

Math: loss = mean_b [ (sum_e w_be * P[b,i_e,:] @ D @ P[b,j_e,:]) / max(sum_e w_be, 1e-8) ]

Reformulation:
    G_b = (P_b @ D) @ P_b^T            (two 256^3 matmuls on the PE, bf16)
    sum_e w_e * P[b,i_e,:] @ D @ P[b,j_e,:] = sum_e w_e * G_b[i_e, j_e]

Per-edge access path:
    Both G tables of a batch pair spill to one DRAM table viewed as
    [65536, 2] f32; one indirect DMA per pair (hardware dynamic DGE)
    fetches 2 consecutive f32 per edge at 16-bit offset
    idx = b2*32768 + (f>>1), f = 256*i + j. The odd/even lane select is
    folded into host-interleaved weights (w at lane f&1, 0 at the other),
    so prod-and-reduce needs one multiply + one reduce per batch.

Sharding: data-parallel over batch: 8 NeuronCores x 8 batches. Each core
emits per-sample partial sums (sum w*g and sum w per batch); the host
performs the final divide + mean (the all-reduce of the sharding hint).
"""

from contextlib import ExitStack

import ml_dtypes
import numpy as np

import concourse.bacc as bacc
import concourse.bass as bass
import concourse.mybir as mybir
import concourse.tile as tile
from concourse.bass_utils import run_bass_kernel_spmd

B, N, E = 64, 256, 8192
NCORES = 8
BPC = B // NCORES  # batches per core
NPAIR = BPC // 2
Q = E // 128  # edges per partition (64)

f32 = mybir.dt.float32
bf16 = mybir.dt.bfloat16
i32 = mybir.dt.int32


def _build_bass():
    nc = bacc.Bacc("TRN2", target_bir_lowering=False, debug=False)

    # pt[t, p, kc, b2, i] = P[2t+b2, i, kc*128+p]
    pt_in = nc.dram_tensor("pt", [NPAIR, 128, 2, 2, N], bf16, kind="ExternalInput")
    d_in = nc.dram_tensor("derr", [128, 2, N], bf16, kind="ExternalInput")
    # eidx[p, t, b2, q] = b2*32768 + (256*i + j)>>1
    ei_in = nc.dram_tensor("eidx", [128, NPAIR, 2, Q], i32, kind="ExternalInput")
    # ew2[p, t, b2, q, l] = w if l == (256*i + j)&1 else 0
    ew_in = nc.dram_tensor("ew2", [128, NPAIR, 2, Q, 2], f32, kind="ExternalInput")
    out = nc.dram_tensor("out", [1, 2 * BPC], f32, kind="ExternalOutput")

    with tile.TileContext(nc) as tc, ExitStack() as ctx:
        const_pool = ctx.enter_context(tc.tile_pool(name="const", bufs=1))
        pt_pool = ctx.enter_context(tc.tile_pool(name="pt", bufs=3))
        qt_pool = ctx.enter_context(tc.tile_pool(name="qt", bufs=2))
        g_pool = ctx.enter_context(tc.tile_pool(name="g", bufs=3))
        e_pool = ctx.enter_context(tc.tile_pool(name="edges", bufs=3))
        psum_pool = ctx.enter_context(tc.tile_pool(name="ps", bufs=3, space="PSUM"))
        psum1_pool = ctx.enter_context(tc.tile_pool(name="ps1", bufs=1, space="PSUM"))
        dram_pool = ctx.enter_context(tc.tile_pool(name="dram", bufs=4, space="DRAM"))

        # inputs on the scalar queue: the sync queue carries only spills, so
        # a spill's data never waits behind bulk input transfers
        d_sb = const_pool.tile([128, 2, N], bf16)
        nc.scalar.dma_start(d_sb[:], d_in[:])
        eidx_sb = const_pool.tile([128, NPAIR, 2, Q], i32)
        ew2_sb = const_pool.tile([128, NPAIR, 2, Q, 2], f32)
        ones_sb = const_pool.tile([128, 1], f32)
        nc.vector.memset(ones_sb[:], 1.0)
        # per-batch partials: cols [0,BPC) = sum(w*g), cols [BPC,2*BPC) = sum(w)
        red_sb = const_pool.tile([128, 2 * BPC], f32)

        pending = None  # (gsel tile, t) awaiting reduce — one pair behind

        def flush_pending():
            nonlocal pending
            if pending is None:
                return
            gsel, t = pending
            prod = e_pool.tile([128, 2, Q, 2], f32, tag="prod")
            nc.vector.tensor_tensor(
                out=prod[:], in0=gsel[:], in1=ew2_sb[:, t], op=mybir.AluOpType.mult
            )
            for b2 in range(2):
                b = 2 * t + b2
                nc.vector.tensor_reduce(
                    out=red_sb[:, b : b + 1],
                    in_=prod[:, b2].rearrange("p a b -> p (a b)"),
                    axis=mybir.AxisListType.X,
                    op=mybir.AluOpType.add,
                )
            pending = None

        for t in range(NPAIR):
            # ---- load P^T for a batch pair
            pt2 = pt_pool.tile([128, 2, 2, N], bf16)
            nc.scalar.dma_start(pt2[:], pt_in[t])
            if t == 0:
                # after d/pt0 in the scalar queue; before any edge reads
                nc.scalar.dma_start(eidx_sb[:], ei_in[:])
                nc.scalar.dma_start(ew2_sb[:], ew_in[:])

            # ---- QT[n, (b2, i)] = sum_k D[k, n] * PT[k, (b2, i)]
            qt_sb = qt_pool.tile([128, 2, 2, N], bf16)  # (ncx, b2, i)
            for ncx in range(2):
                qt_ps = psum_pool.tile([128, 2, N], f32, tag="qtps")
                for kc in range(2):
                    nc.tensor.matmul(
                        qt_ps[:].rearrange("p a b -> p (a b)"),
                        lhsT=d_sb[:, kc, ncx * 128 : (ncx + 1) * 128],
                        rhs=pt2[:, kc, :, :].rearrange("p a b -> p (a b)"),
                        start=(kc == 0),
                        stop=(kc == 1),
                    )
                nc.scalar.copy(qt_sb[:, ncx], qt_ps[:])

            g_d = dram_pool.tile([2, 2, 128, N], f32, tag="gd")  # (b2, c, p, j)
            for b2 in range(2):
                # ---- G[(ic), j] = sum_n QT[n, i] * PT[n, j]
                g_ps = psum_pool.tile([128, 2, N], f32, tag="gps")  # (ic, j)
                for ic in range(2):
                    for ncx in range(2):
                        nc.tensor.matmul(
                            g_ps[:, ic, :],
                            lhsT=qt_sb[:, ncx, b2, ic * 128 : (ic + 1) * 128],
                            rhs=pt2[:, ncx, b2, :],
                            start=(ncx == 0),
                            stop=(ncx == 1),
                        )
                g_sb = g_pool.tile([128, 2, N], f32)
                if b2 == 0:
                    nc.vector.tensor_copy(g_sb[:], g_ps[:])
                else:
                    nc.scalar.copy(g_sb[:], g_ps[:])
                # sum(w) reduce for this batch: vector has slack here
                b = 2 * t + b2
                nc.vector.tensor_reduce(
                    out=red_sb[:, BPC + b : BPC + b + 1],
                    in_=ew2_sb[:, t, b2].rearrange("p a b -> p (a b)"),
                    axis=mybir.AxisListType.X,
                    op=mybir.AluOpType.add,
                )
                # ---- spill this batch's G into its half of the pair table
                nc.sync.dma_start(
                    g_d[b2].rearrange("c p j -> p c j"), g_sb[:]
                )

            # ---- gather 2 consecutive f32 per edge, both batches in one
            # indirect DMA (16-bit offsets: b2*32768 + f>>1)
            gsel = e_pool.tile([128, 2, Q, 2], f32, tag="gsel")
            nc.gpsimd.indirect_dma_start(
                out=gsel[:].rearrange("p a b c -> p (a b c)"),
                out_offset=None,
                in_=g_d.rearrange("b c p (j2 u) -> (b c p j2) u", u=2),
                in_offset=bass.IndirectOffsetOnAxis(ap=eidx_sb[:, t], axis=0),
            )

            # ---- reduce the PREVIOUS pair (its gather is long done)
            flush_pending()
            pending = (gsel, t)

        flush_pending()

        # ---- cross-partition reduce of all partials in one matmul
        red_ps = psum1_pool.tile([1, 2 * BPC], f32, tag="redps")
        nc.tensor.matmul(
            red_ps[:], lhsT=ones_sb[:], rhs=red_sb[:], start=True, stop=True
        )
        fin = const_pool.tile([1, 2 * BPC], f32)
        nc.vector.tensor_copy(fin[:], red_ps[:])
        nc.sync.dma_start(out[:], fin[:])

    if not nc.is_finalized():
        nc.finalize()
    return nc


_NC_CACHE = {}


def _get_nc():
    if "nc" not in _NC_CACHE:
        _NC_CACHE["nc"] = _build_bass()
    return _NC_CACHE["nc"]


def _prep_in_maps(P, d_error, edge_i, edge_j, edge_w):
    P = np.asarray(P, dtype=np.float32)
    d_error = np.asarray(d_error, dtype=np.float32)
    edge_i = np.asarray(edge_i, dtype=np.int32)
    edge_j = np.asarray(edge_j, dtype=np.int32)
    edge_w = np.asarray(edge_w, dtype=np.float32)

    # P^T pairs: pt[t, p, kc, b2, i] = P[2t+b2, i, kc*128+p]
    PT = np.ascontiguousarray(np.transpose(P, (0, 2, 1)))  # [B, N(k), N(i)]
    PT = PT.reshape(B // 2, 2, 2, 128, N).transpose(0, 3, 2, 1, 4)
    PT = np.ascontiguousarray(PT).astype(ml_dtypes.bfloat16)
    D = np.ascontiguousarray(d_error.reshape(2, 128, N).transpose(1, 0, 2))
    D = D.astype(ml_dtypes.bfloat16)

    # gather index: b2*32768 + f>>1 (pair-table element-pair offset)
    f = (edge_i << 8) | edge_j  # [B, E] int32
    b_off = (np.arange(B, dtype=np.int32) & 1)[:, None] << 15
    idx = b_off | (f >> 1)
    # lane-selected weights: w at lane f&1, 0 at the other
    lane = (f & 1)[..., None]  # [B, E, 1]
    w2 = edge_w[..., None] * (lane == np.arange(2)).astype(np.float32)  # [B, E, 2]

    # edge e = q*128 + p at [p, q]
    # idx -> [p, t, b2, q]
    idx_l = idx.reshape(B // 2, 2, Q, 128).transpose(3, 0, 1, 2)
    idx_l = np.ascontiguousarray(idx_l)
    # w2 -> [p, t, b2, q, l]
    w2_l = w2.reshape(B // 2, 2, Q, 128, 2).transpose(3, 0, 1, 2, 4)
    w2_l = np.ascontiguousarray(w2_l)

    in_maps = []
    for c in range(NCORES):
        sl = slice(c * NPAIR, (c + 1) * NPAIR)
        in_maps.append(
            {
                "pt": np.ascontiguousarray(PT[sl]),
                "derr": D,
                "eidx": np.ascontiguousarray(idx_l[:, sl]),
                "ew2": np.ascontiguousarray(w2_l[:, sl]),
            }
        )
    return in_maps


def run(P, d_error, edge_i, edge_j, edge_w, trace=False):
    """Run on 8 cores; returns (loss_scalar, BassKernelResults)."""
    nc = _get_nc()
    in_maps = _prep_in_maps(P, d_error, edge_i, edge_j, edge_w)
    res = run_bass_kernel_spmd(
        nc, in_maps, core_ids=list(range(NCORES)), trace=trace
    )
    # host-side all-reduce: loss = mean_b( sl_b / max(sw_b, 1e-8) )
    acc = 0.0
    for r in res.results:
        part = np.asarray(r["out"], dtype=np.float64).reshape(2 * BPC)
        sl, sw = part[:BPC], part[BPC:]
        acc += float(np.sum(sl / np.maximum(sw, 1e-8)))
    loss = np.float32(acc / B)
    return loss, res


def kernel(P, d_error, edge_i, edge_j, edge_w):
    loss, _ = run(P, d_error, edge_i, edge_j, edge_w, trace=False)
    return np.asarray(loss, dtype=np.float32)


# revision 19
# speedup vs baseline: 1.0998x; 1.0998x over previous
"""ErrorAwareEdgeLoss Trainium2 kernel.

Math: loss = mean_b [ (sum_e w_be * P[b,i_e,:] @ D @ P[b,j_e,:]) / max(sum_e w_be, 1e-8) ]

Reformulation:
    G_b = (P_b @ D) @ P_b^T            (two 256^3 matmuls on the PE, bf16)
    sum_e w_e * P[b,i_e,:] @ D @ P[b,j_e,:] = sum_e w_e * G_b[i_e, j_e]

Per-edge access path:
    Both G tables of a batch pair spill to one DRAM table in bf16; viewed
    as f32 each element packs two adjacent bf16 values. One indirect DMA
    per pair (hardware dynamic DGE, per-element 16-bit offsets
    idx = b2*32768 + (f>>1), f = 256*i + j, one offset per gathered f32)
    fetches every edge's value. The odd/even bf16 lane select is folded
    into host-interleaved bf16 weights (w at lane f&1, 0 at the other), so
    prod-and-reduce is one multiply + two reduces per pair.

Sharding: data-parallel over batch: 8 NeuronCores x 8 batches. Each core
emits per-sample partial sums (sum w*g and sum w per batch); the host
performs the final divide + mean (the all-reduce of the sharding hint).
"""

from contextlib import ExitStack

import ml_dtypes
import numpy as np

import concourse.bacc as bacc
import concourse.bass as bass
import concourse.mybir as mybir
import concourse.tile as tile
from concourse.bass_utils import run_bass_kernel_spmd

B, N, E = 64, 256, 8192
NCORES = 8
BPC = B // NCORES  # batches per core
NPAIR = BPC // 2
Q = E // 128  # edges per partition (64)

f32 = mybir.dt.float32
bf16 = mybir.dt.bfloat16
i32 = mybir.dt.int32


def _build_bass():
    nc = bacc.Bacc("TRN2", target_bir_lowering=False, debug=False)

    # pt[t, p, kc, b2, i] = P[2t+b2, i, kc*128+p]
    pt_in = nc.dram_tensor("pt", [NPAIR, 128, 2, 2, N], bf16, kind="ExternalInput")
    d_in = nc.dram_tensor("derr", [128, 2, N], bf16, kind="ExternalInput")
    # eidx[p, t, b2, q] = b2*32768 + (256*i + j)>>1
    ei_in = nc.dram_tensor("eidx", [128, NPAIR, 2, Q], i32, kind="ExternalInput")
    # ew2[p, t, b2, 2q+l] = bf16(w) if l == (256*i + j)&1 else 0
    ew_in = nc.dram_tensor("ew2", [128, NPAIR, 2, 2 * Q], bf16, kind="ExternalInput")
    out = nc.dram_tensor("out", [1, 2 * BPC], f32, kind="ExternalOutput")

    with tile.TileContext(nc) as tc, ExitStack() as ctx:
        const_pool = ctx.enter_context(tc.tile_pool(name="const", bufs=1))
        pt_pool = ctx.enter_context(tc.tile_pool(name="pt", bufs=3))
        qt_pool = ctx.enter_context(tc.tile_pool(name="qt", bufs=2))
        g_pool = ctx.enter_context(tc.tile_pool(name="g", bufs=3))
        e_pool = ctx.enter_context(tc.tile_pool(name="edges", bufs=3))
        psum_pool = ctx.enter_context(tc.tile_pool(name="ps", bufs=3, space="PSUM"))
        psum1_pool = ctx.enter_context(tc.tile_pool(name="ps1", bufs=1, space="PSUM"))
        dram_pool = ctx.enter_context(tc.tile_pool(name="dram", bufs=4, space="DRAM"))

        # inputs on the scalar queue: the sync queue carries only spills, so
        # a spill's data never waits behind bulk input transfers
        d_sb = const_pool.tile([128, 2, N], bf16)
        nc.scalar.dma_start(d_sb[:], d_in[:])
        eidx_sb = const_pool.tile([128, NPAIR, 2, Q], i32)
        ew2_sb = const_pool.tile([128, NPAIR, 2, 2 * Q], bf16)
        ones_sb = const_pool.tile([128, 1], f32)
        nc.vector.memset(ones_sb[:], 1.0)
        # per-batch partials: cols [0,BPC) = sum(w*g), cols [BPC,2*BPC) = sum(w)
        red_sb = const_pool.tile([128, 2 * BPC], f32)

        pending = None  # (gsel tile, t) awaiting reduce — one pair behind

        def flush_pending():
            nonlocal pending
            if pending is None:
                return
            gsel, t = pending
            prod = e_pool.tile([128, 2, 2 * Q], f32, tag="prod")
            nc.vector.tensor_tensor(
                out=prod[:],
                in0=gsel[:].bitcast(bf16),
                in1=ew2_sb[:, t],
                op=mybir.AluOpType.mult,
            )
            for b2 in range(2):
                b = 2 * t + b2
                nc.vector.tensor_reduce(
                    out=red_sb[:, b : b + 1],
                    in_=prod[:, b2],
                    axis=mybir.AxisListType.X,
                    op=mybir.AluOpType.add,
                )
            pending = None

        for t in range(NPAIR):
            # ---- load P^T for a batch pair
            pt2 = pt_pool.tile([128, 2, 2, N], bf16)
            nc.scalar.dma_start(pt2[:], pt_in[t])
            if t == 0:
                # after d/pt0 in the scalar queue; before any edge reads
                nc.scalar.dma_start(eidx_sb[:], ei_in[:])
                nc.scalar.dma_start(ew2_sb[:], ew_in[:])

            # ---- QT[n, (b2, i)] = sum_k D[k, n] * PT[k, (b2, i)]
            qt_sb = qt_pool.tile([128, 2, 2, N], bf16)  # (ncx, b2, i)
            for ncx in range(2):
                qt_ps = psum_pool.tile([128, 2, N], f32, tag="qtps")
                for kc in range(2):
                    nc.tensor.matmul(
                        qt_ps[:].rearrange("p a b -> p (a b)"),
                        lhsT=d_sb[:, kc, ncx * 128 : (ncx + 1) * 128],
                        rhs=pt2[:, kc, :, :].rearrange("p a b -> p (a b)"),
                        start=(kc == 0),
                        stop=(kc == 1),
                    )
                nc.scalar.copy(qt_sb[:, ncx], qt_ps[:])

            g_d = dram_pool.tile([2, 2, 128, N], bf16, tag="gd")  # (b2, c, p, j)
            for b2 in range(2):
                b = 2 * t + b2
                # ---- G[(ic), j] = sum_n QT[n, i] * PT[n, j]
                g_ps = psum_pool.tile([128, 2, N], f32, tag="gps")  # (ic, j)
                for ic in range(2):
                    for ncx in range(2):
                        nc.tensor.matmul(
                            g_ps[:, ic, :],
                            lhsT=qt_sb[:, ncx, b2, ic * 128 : (ic + 1) * 128],
                            rhs=pt2[:, ncx, b2, :],
                            start=(ncx == 0),
                            stop=(ncx == 1),
                        )
                g_sb = g_pool.tile([128, 2, N], bf16)
                if b2 == 0:
                    nc.vector.tensor_copy(g_sb[:], g_ps[:])
                else:
                    nc.scalar.copy(g_sb[:], g_ps[:])
                # sum(w) reduce for this batch: vector has slack here
                nc.vector.tensor_reduce(
                    out=red_sb[:, BPC + b : BPC + b + 1],
                    in_=ew2_sb[:, t, b2],
                    axis=mybir.AxisListType.X,
                    op=mybir.AluOpType.add,
                )
                # ---- spill this batch's G into its half of the pair table
                nc.sync.dma_start(g_d[b2].rearrange("c p j -> p c j"), g_sb[:])

            # ---- gather one packed-f32 (two bf16 lanes) per edge, both
            # batches in one indirect DMA (16-bit offsets, one per element)
            gsel = e_pool.tile([128, 2, Q], f32, tag="gsel")
            nc.gpsimd.indirect_dma_start(
                out=gsel[:].rearrange("p a b -> p (a b)"),
                out_offset=None,
                in_=g_d[:]
                .bitcast(f32)
                .rearrange("b c p (j2 u) -> (b c p j2) u", u=1),
                in_offset=bass.IndirectOffsetOnAxis(
                    ap=eidx_sb[:, t].rearrange("p a b -> p (a b)"), axis=0
                ),
            )

            # ---- reduce the PREVIOUS pair (its gather is long done)
            flush_pending()
            pending = (gsel, t)

        flush_pending()

        # ---- cross-partition reduce of all partials in one matmul
        red_ps = psum1_pool.tile([1, 2 * BPC], f32, tag="redps")
        nc.tensor.matmul(
            red_ps[:], lhsT=ones_sb[:], rhs=red_sb[:], start=True, stop=True
        )
        fin = const_pool.tile([1, 2 * BPC], f32)
        nc.vector.tensor_copy(fin[:], red_ps[:])
        nc.sync.dma_start(out[:], fin[:])

    if not nc.is_finalized():
        nc.finalize()
    return nc


_NC_CACHE = {}


def _get_nc():
    if "nc" not in _NC_CACHE:
        _NC_CACHE["nc"] = _build_bass()
    return _NC_CACHE["nc"]


def _prep_in_maps(P, d_error, edge_i, edge_j, edge_w):
    P = np.asarray(P, dtype=np.float32)
    d_error = np.asarray(d_error, dtype=np.float32)
    edge_i = np.asarray(edge_i, dtype=np.int32)
    edge_j = np.asarray(edge_j, dtype=np.int32)
    edge_w = np.asarray(edge_w, dtype=np.float32)

    # P^T pairs: pt[t, p, kc, b2, i] = P[2t+b2, i, kc*128+p]
    PT = np.ascontiguousarray(np.transpose(P, (0, 2, 1)))  # [B, N(k), N(i)]
    PT = PT.reshape(B // 2, 2, 2, 128, N).transpose(0, 3, 2, 1, 4)
    PT = np.ascontiguousarray(PT).astype(ml_dtypes.bfloat16)
    D = np.ascontiguousarray(d_error.reshape(2, 128, N).transpose(1, 0, 2))
    D = D.astype(ml_dtypes.bfloat16)

    # gather index: b2*32768 + f>>1 (packed-f32 offset in the pair table)
    f = (edge_i << 8) | edge_j  # [B, E] int32
    b_off = (np.arange(B, dtype=np.int32) & 1)[:, None] << 15
    idx = b_off | (f >> 1)
    # lane-selected bf16 weights: w at lane f&1, 0 at the other
    lane = (f & 1)[..., None]  # [B, E, 1]
    w2 = edge_w[..., None] * (lane == np.arange(2)).astype(np.float32)
    w2 = w2.astype(ml_dtypes.bfloat16)  # [B, E, 2]

    # edge e = q*128 + p at [p, q]
    # idx -> [p, t, b2, q]
    idx_l = idx.reshape(B // 2, 2, Q, 128).transpose(3, 0, 1, 2)
    idx_l = np.ascontiguousarray(idx_l)
    # w2 -> [p, t, b2, (q, l)]
    w2_l = w2.reshape(B // 2, 2, Q, 128, 2).transpose(3, 0, 1, 2, 4)
    w2_l = np.ascontiguousarray(w2_l).reshape(128, B // 2, 2, 2 * Q)

    in_maps = []
    for c in range(NCORES):
        sl = slice(c * NPAIR, (c + 1) * NPAIR)
        in_maps.append(
            {
                "pt": np.ascontiguousarray(PT[sl]),
                "derr": D,
                "eidx": np.ascontiguousarray(idx_l[:, sl]),
                "ew2": np.ascontiguousarray(w2_l[:, sl]),
            }
        )
    return in_maps


def run(P, d_error, edge_i, edge_j, edge_w, trace=False):
    """Run on 8 cores; returns (loss_scalar, BassKernelResults)."""
    nc = _get_nc()
    in_maps = _prep_in_maps(P, d_error, edge_i, edge_j, edge_w)
    res = run_bass_kernel_spmd(
        nc, in_maps, core_ids=list(range(NCORES)), trace=trace
    )
    # host-side all-reduce: loss = mean_b( sl_b / max(sw_b, 1e-8) )
    acc = 0.0
    for r in res.results:
        part = np.asarray(r["out"], dtype=np.float64).reshape(2 * BPC)
        sl, sw = part[:BPC], part[BPC:]
        acc += float(np.sum(sl / np.maximum(sw, 1e-8)))
    loss = np.float32(acc / B)
    return loss, res


def kernel(P, d_error, edge_i, edge_j, edge_w):
    loss, _ = run(P, d_error, edge_i, edge_j, edge_w, trace=False)
    return np.asarray(loss, dtype=np.float32)
